# revision 1
# baseline (speedup 1.0000x reference)
"""Trainium2 Bass kernel for CausalWanSelfAttention (KV-cache-bias attention).

Math: the reference's disjoint-segment attention + LSE merge is exactly
global softmax with a per-key bias b_l (log 0.1 on keys in
[frame_seqlen, current_block_start)).  exp needs no max-subtraction
(scores ~ N(0,1), max ~ 6), so out = (E @ V) / (1^T E) with
E = exp(scale*S + b_l) — the bias folds into the ACT exp as a
per-partition bias (partition = key index within the 128-chunk).

Sharding: 24 units = (head h in 0..11, q-half in {0,1}), 3 units per core.
Each unit: 1024 queries x 1 head x all 8192 keys, 64 key chunks of 128.

Device layout per unit (matmuls bf16, accumulate fp32 PSUM; all matmuls
stream 512 q-columns so PE runs long back-to-back bursts with only 3
ldweights per chunk — the v1 kernel's 8 stationary loads per chunk made
phase B ldweights-bound on HW):
  A:    S^T[l 128, q 1024] = kt-chunk^T @ qt          (1 ldw + 2 MM N=512)
  exp:  E = exp(S^T * scale + bias_l) bf16            (1 ACT instr)
  B:    O^T[d 128, q 1024] += v-chunk^T @ E           (1 ldw + 2 MM)
  norm: n[1, q 1024]      += ones^T @ E               (1 ldw + 2 MM)
Final divide by n and the [d,q]->[q,d] transpose happen host-side on the
fp32 partials (exact).
"""

import math
import sys

for _p in ("/opt/trn_rl_repo",):
    if _p not in sys.path:
        sys.path.insert(0, _p)

import numpy as np
import ml_dtypes

import concourse.bass as bass
import concourse.mybir as mybir
import concourse.tile as tile
from concourse import bacc
from concourse.bass_utils import run_bass_kernel_spmd

BF16 = mybir.dt.bfloat16
F32 = mybir.dt.float32
NP_BF16 = ml_dtypes.bfloat16

B, LQ, LK, H, D = 1, 2048, 8192, 12, 128
N_CORES = 8
UNITS_PER_CORE = 3          # 24 units = 12 heads x 2 q-halves
QSPAN = 1024                # queries per unit
NLC = LK // 128             # 64 key chunks of 128
SCALE = 1.0 / math.sqrt(D)

_CACHED = None
ABLATE = "base"   # timing experiments only; "base" is the real kernel
TIME_LOOP = 1     # timing experiments only: hardware-loop the body N times


def _build_program():
    nc = bacc.Bacc("TRN2", target_bir_lowering=False, debug=False,
                   enable_asserts=False)

    qt_d = nc.dram_tensor("qt", [UNITS_PER_CORE, 128, QSPAN], BF16,
                          kind="ExternalInput")
    kt_d = nc.dram_tensor("kt", [UNITS_PER_CORE, 128, LK], BF16,
                          kind="ExternalInput")
    vl_d = nc.dram_tensor("vl", [UNITS_PER_CORE, LK, 128], BF16,
                          kind="ExternalInput")
    bias_d = nc.dram_tensor("bias", [128, NLC], F32, kind="ExternalInput")
    ot_d = nc.dram_tensor("ot", [UNITS_PER_CORE, 128, QSPAN], F32,
                          kind="ExternalOutput")
    nm_d = nc.dram_tensor("nm", [UNITS_PER_CORE, 1, QSPAN], F32,
                          kind="ExternalOutput")

    qt_ap = qt_d.ap()
    kt_ap = kt_d.ap()
    # [u, (c p), d] -> [u, p, c, d]: partition = key index within chunk
    vl_ap = vl_d.ap().rearrange("u (c p) d -> u p c d", p=128)
    bias_ap = bias_d.ap()
    ot_ap = ot_d.ap()
    nm_ap = nm_d.ap()

    with tile.TileContext(nc) as tc:
        with (
            tc.tile_pool(name="kt_pool", bufs=2) as kt_pool,
            tc.tile_pool(name="vl_pool", bufs=2) as vl_pool,
            tc.tile_pool(name="qt_pool", bufs=2) as qt_pool,
            tc.tile_pool(name="cn_pool", bufs=1) as cn_pool,
            tc.tile_pool(name="e_pool", bufs=4) as e_pool,
            tc.tile_pool(name="ob_pool", bufs=2) as ob_pool,
            tc.tile_pool(name="s_pool", bufs=2, space="PSUM") as s_pool,
            tc.tile_pool(name="o_pool", bufs=1, space="PSUM") as o_pool,
            tc.tile_pool(name="n_pool", bufs=1, space="PSUM") as n_pool,
        ):
            bias_t = cn_pool.tile([128, NLC], F32, name="bias_t")
            nc.sync.dma_start(out=bias_t[:], in_=bias_ap)
            ones_t = cn_pool.tile([128, 1], BF16, name="ones_t")
            nc.vector.memset(ones_t[:], 1.0)

            import contextlib
            loop_cm = (tc.For_i(0, TIME_LOOP, 1) if TIME_LOOP > 1
                       else contextlib.nullcontext())

            loaded = {}

            def load_unit(u):
                # qt first (every chunk needs it), then k/v interleaved in
                # eighths so chunk 0's compute starts after ~1/8 of the load
                qt = qt_pool.tile([128, QSPAN], BF16, name=f"qt_u{u}", tag="qt")
                nc.sync.dma_start(out=qt[:], in_=qt_ap[u])
                kt = kt_pool.tile([128, LK], BF16, name=f"kt_u{u}", tag="kt")
                vl = vl_pool.tile([128, NLC, 128], BF16,
                                  name=f"vl_u{u}", tag="vl")
                for eighth in range(8):
                    slk = bass.ts(eighth, LK // 8)
                    nc.sync.dma_start(out=kt[:, slk], in_=kt_ap[u][:, slk])
                    slv = bass.ts(eighth, NLC // 8)
                    nc.sync.dma_start(out=vl[:, slv, :], in_=vl_ap[u][:, slv, :])
                loaded[u] = (kt, vl, qt)

            NG = UNITS_PER_CORE * NLC

            with loop_cm:
                load_unit(0)
                # One global software-pipelined chunk stream across all
                # units: emit A(g) before B(g-1) so PE's in-order queue
                # always has independent work while ACT runs exp(g-1), and
                # the next unit's A-phase fills the previous unit's drain.
                # Norm: DVE pre-reduces E pairs -> quads so PE streams only
                # NLC/4 norm matmuls per unit, 2 chunks delayed so the DVE
                # adds never stall the PE queue.  PSUM accumulators (ot/nm,
                # single-buffered) are allocated lazily at first write so
                # the pool rotation lands after the previous unit's
                # evacuation instr is emitted.
                cur, ot_t, nm_t = {}, {}, {}
                etiles, ptiles, qtiles = {}, {}, {}
                for g in range(NG + 4):
                    if g < NG:
                        ug, cg = g // NLC, g % NLC
                        if cg == 0:
                            cur[ug] = loaded.pop(ug)
                        kt, vl, qt = cur[ug]
                        s = s_pool.tile([128, QSPAN], F32)
                        for half in range(2):
                            sl = bass.ts(half, 512)
                            nc.tensor.matmul(
                                s[:, sl], lhsT=kt[:, bass.ts(cg, 128)],
                                rhs=qt[:, sl], start=True, stop=True)
                        e = e_pool.tile([128, QSPAN], BF16)
                        nc.scalar.activation(
                            e[:], s[:], mybir.ActivationFunctionType.Exp,
                            bias=bias_t[:, cg:cg + 1], scale=SCALE)
                        etiles[g] = e
                        if cg == 8 and ug + 1 < UNITS_PER_CORE:
                            load_unit(ug + 1)  # prefetch next unit's inputs
                    d = g - 1               # chunk whose B phase is due
                    if 0 <= d < NG:
                        ud, dl = d // NLC, d % NLC
                        if dl == 0:
                            ot_t[ud] = o_pool.tile([128, QSPAN], F32,
                                                   name=f"ot_u{ud}", tag="ot")
                        e = etiles[d]
                        for half in range(2):
                            sl = bass.ts(half, 512)
                            nc.tensor.matmul(
                                ot_t[ud][:, sl], lhsT=cur[ud][1][:, dl, :],
                                rhs=e[:, sl],
                                start=(dl == 0), stop=(dl == NLC - 1))
                        if dl % 2 == 1:
                            pp = e_pool.tile([128, QSPAN], BF16,
                                             tag="pp", name=f"pp_{d}")
                            nc.vector.tensor_add(
                                pp[:], etiles.pop(d - 1)[:], etiles[d][:])
                            ptiles[d // 2] = pp
                        if dl % 4 == 3:
                            qq = e_pool.tile([128, QSPAN], BF16,
                                             tag="qq", name=f"qq_{d}")
                            nc.vector.tensor_add(
                                qq[:], ptiles.pop(d // 2 - 1)[:],
                                ptiles.pop(d // 2)[:])
                            qtiles[d // 4] = qq
                        if dl == NLC - 1:
                            etiles.pop(d)
                            ot = ot_t.pop(ud)
                            ot_sb = ob_pool.tile([128, QSPAN], F32,
                                                 name=f"otsb_u{ud}",
                                                 tag="otsb")
                            nc.vector.tensor_scalar_add(ot_sb[:], ot[:], 0.0)
                            nc.sync.dma_start(out=ot_ap[ud], in_=ot_sb[:])
                    n = g - 3               # norm quad due, 2 chunks delayed
                    if 0 <= n < NG:
                        un, nl = n // NLC, n % NLC
                        if nl % 4 == 3:
                            if nl == 3:
                                nm_t[un] = n_pool.tile([128, QSPAN], F32,
                                                       name=f"nm_u{un}",
                                                       tag="nm")
                            qq = qtiles.pop(n // 4)
                            for half in range(2):
                                sl = bass.ts(half, 512)
                                nc.tensor.matmul(
                                    nm_t[un][0:1, sl], lhsT=ones_t[:],
                                    rhs=qq[:, sl],
                                    start=(nl == 3), stop=(nl == NLC - 1))
                            if nl == NLC - 1:
                                nm = nm_t.pop(un)
                                nm_sb = ob_pool.tile([1, QSPAN], F32,
                                                     name=f"nmsb_u{un}",
                                                     tag="nmsb")
                                nc.vector.tensor_scalar_add(
                                    nm_sb[:], nm[0:1, :], 0.0)
                                nc.sync.dma_start(out=nm_ap[un], in_=nm_sb[:])

    nc.compile()
    return nc


def _get_program():
    global _CACHED
    if _CACHED is None:
        _CACHED = _build_program()
    return _CACHED


def _host_prep(q, k, v, frame_seqlen, current_block_start):
    fs = max(0, min(int(frame_seqlen), LK))
    bs = max(0, min(int(current_block_start), LK))
    logw = np.zeros(LK, np.float32)
    logw[fs:bs] = math.log(0.1)
    bias = np.ascontiguousarray(logw.reshape(NLC, 128).T)  # [128, NLC]

    q = np.asarray(q, dtype=np.float32)
    k = np.asarray(k, dtype=np.float32)
    v = np.asarray(v, dtype=np.float32)

    qT = np.ascontiguousarray(q[0].transpose(1, 2, 0)).astype(NP_BF16)  # [H,128,LQ]
    kT = np.ascontiguousarray(k[0].transpose(1, 2, 0)).astype(NP_BF16)  # [H,128,LK]
    vL = np.ascontiguousarray(v[0].transpose(1, 0, 2)).astype(NP_BF16)  # [H,LK,128]

    in_maps = []
    for i in range(N_CORES):
        units = [3 * i + uu for uu in range(UNITS_PER_CORE)]
        heads = [g // 2 for g in units]
        qhs = [g % 2 for g in units]
        in_maps.append({
            "qt": np.ascontiguousarray(
                np.stack([qT[h, :, qh * QSPAN:(qh + 1) * QSPAN]
                          for h, qh in zip(heads, qhs)])),
            "kt": np.ascontiguousarray(np.stack([kT[h] for h in heads])),
            "vl": np.ascontiguousarray(np.stack([vL[h] for h in heads])),
            "bias": bias,
        })
    return in_maps


def _assemble(results):
    out = np.empty((B, LQ, H, D), np.float32)
    for i in range(N_CORES):
        ot = results[i]["ot"]   # [3, 128, 1024] unnormalized O^T
        nm = results[i]["nm"][:, 0]   # [3, 1024]
        for uu in range(UNITS_PER_CORE):
            g = 3 * i + uu
            h, qh = g // 2, g % 2
            out[0, qh * QSPAN:(qh + 1) * QSPAN, h, :] = (
                ot[uu] / nm[uu][None, :]).T
    return out


def kernel(q, k, v, frame_seqlen, current_block_start):
    nc = _get_program()
    in_maps = _host_prep(q, k, v, frame_seqlen, current_block_start)
    res = run_bass_kernel_spmd(nc, in_maps, core_ids=list(range(N_CORES)))
    return _assemble(res.results)



# revision 6
# speedup vs baseline: 1.9451x; 1.9451x over previous
"""Trainium2 Bass kernel for CausalWanSelfAttention (KV-cache-bias attention).

Math: the reference's disjoint-segment attention + LSE merge is exactly
global softmax with a per-key bias b_l (log 0.1 on keys in
[frame_seqlen, current_block_start)).  exp needs no max-subtraction
(scores ~ N(0,1), max ~ 6), so out = (E @ V) / (1^T E) with
E = exp(scale*S + b_l) — the bias folds into the ACT exp as a
per-partition bias (partition = key index within the 128-chunk).

Sharding: 24 units = (head h in 0..11, q-half in {0,1}), 3 units per core.
Each unit: 1024 queries x 1 head x all 8192 keys, 64 key chunks of 128.

Device layout per unit (matmuls bf16, accumulate fp32 PSUM; all matmuls
stream 512 q-columns so PE runs long back-to-back bursts with only 3
ldweights per chunk — the v1 kernel's 8 stationary loads per chunk made
phase B ldweights-bound on HW):
  A:    S^T[l 128, q 1024] = kt-chunk^T @ qt          (1 ldw + 2 MM N=512)
  exp:  E = exp(S^T * scale + bias_l) bf16            (1 ACT instr)
  B:    O^T[d 128, q 1024] += v-chunk^T @ E           (1 ldw + 2 MM)
  norm: n[1, q 1024]      += ones^T @ E               (1 ldw + 2 MM)
Final divide by n and the [d,q]->[q,d] transpose happen host-side on the
fp32 partials (exact).
"""

import math
import sys

for _p in ("/opt/trn_rl_repo",):
    if _p not in sys.path:
        sys.path.insert(0, _p)

import numpy as np
import ml_dtypes

import concourse.bass as bass
import concourse.mybir as mybir
import concourse.tile as tile
from concourse import bacc
from concourse.bass_utils import run_bass_kernel_spmd

BF16 = mybir.dt.bfloat16
F32 = mybir.dt.float32
NP_BF16 = ml_dtypes.bfloat16

B, LQ, LK, H, D = 1, 2048, 8192, 12, 128
N_CORES = 8
UNITS_PER_CORE = 3          # 24 units = 12 heads x 2 q-halves
QSPAN = 1024                # queries per unit
NLC = LK // 128             # 64 key chunks of 128
SCALE = 1.0 / math.sqrt(D)

_CACHED = None
ABLATE = "base"   # timing experiments only; "base" is the real kernel
TIME_LOOP = 1     # timing experiments only: hardware-loop the body N times


def _build_program():
    nc = bacc.Bacc("TRN2", target_bir_lowering=False, debug=False,
                   enable_asserts=False)

    qt_d = nc.dram_tensor("qt", [UNITS_PER_CORE, 128, QSPAN], BF16,
                          kind="ExternalInput")
    kt_d = nc.dram_tensor("kt", [UNITS_PER_CORE, 128, LK], BF16,
                          kind="ExternalInput")
    vl_d = nc.dram_tensor("vl", [UNITS_PER_CORE, LK, 128], BF16,
                          kind="ExternalInput")
    bias_d = nc.dram_tensor("bias", [128, NLC], F32, kind="ExternalInput")
    ot_d = nc.dram_tensor("ot", [UNITS_PER_CORE, 128, QSPAN], F32,
                          kind="ExternalOutput")
    nm_d = nc.dram_tensor("nm", [UNITS_PER_CORE, 1, QSPAN], F32,
                          kind="ExternalOutput")

    qt_ap = qt_d.ap()
    kt_ap = kt_d.ap()
    # [u, (c p), d] -> [u, p, c, d]: partition = key index within chunk
    vl_ap = vl_d.ap().rearrange("u (c p) d -> u p c d", p=128)
    bias_ap = bias_d.ap()
    ot_ap = ot_d.ap()
    nm_ap = nm_d.ap()

    with tile.TileContext(nc) as tc:
        with (
            tc.tile_pool(name="kt_pool", bufs=2) as kt_pool,
            tc.tile_pool(name="vl_pool", bufs=2) as vl_pool,
            tc.tile_pool(name="qt_pool", bufs=2) as qt_pool,
            tc.tile_pool(name="cn_pool", bufs=1) as cn_pool,
            tc.tile_pool(name="e_pool", bufs=4) as e_pool,
            tc.tile_pool(name="ob_pool", bufs=2) as ob_pool,
            tc.tile_pool(name="s_pool", bufs=2, space="PSUM") as s_pool,
            tc.tile_pool(name="o_pool", bufs=1, space="PSUM") as o_pool,
            tc.tile_pool(name="n_pool", bufs=1, space="PSUM") as n_pool,
        ):
            bias_t = cn_pool.tile([128, NLC], F32, name="bias_t")
            nc.sync.dma_start(out=bias_t[:], in_=bias_ap)
            ones_t = cn_pool.tile([128, 1], BF16, name="ones_t")
            nc.vector.memset(ones_t[:], 1.0)

            import contextlib
            loop_cm = (tc.For_i(0, TIME_LOOP, 1) if TIME_LOOP > 1
                       else contextlib.nullcontext())

            # ablation switches (timing experiments only)
            do_exp = ABLATE not in ("noexp", "empty")
            do_b = ABLATE not in ("nob", "empty")
            do_any = ABLATE != "empty"
            dummy_e = None
            if not do_exp and do_any:
                dummy_e = cn_pool.tile([128, QSPAN], BF16, name="dummy_e")
                nc.vector.memset(dummy_e[:], 0.001)

            loaded = {}

            def load_unit(u):
                # qt first (every chunk needs it), then k/v interleaved in
                # eighths so chunk 0's compute starts after ~1/8 of the load
                qt = qt_pool.tile([128, QSPAN], BF16, name=f"qt_u{u}", tag="qt")
                nc.sync.dma_start(out=qt[:], in_=qt_ap[u])
                kt = kt_pool.tile([128, LK], BF16, name=f"kt_u{u}", tag="kt")
                vl = vl_pool.tile([128, NLC, 128], BF16,
                                  name=f"vl_u{u}", tag="vl")
                for eighth in range(8):
                    slk = bass.ts(eighth, LK // 8)
                    nc.sync.dma_start(out=kt[:, slk], in_=kt_ap[u][:, slk])
                    slv = bass.ts(eighth, NLC // 8)
                    nc.sync.dma_start(out=vl[:, slv, :], in_=vl_ap[u][:, slv, :])
                loaded[u] = (kt, vl, qt)

            NG = UNITS_PER_CORE * NLC

            with loop_cm:
                if do_any:
                    load_unit(0)
                # One global software-pipelined chunk stream across all
                # units: emit A(g) before B(g-1) so PE's in-order queue
                # always has independent work while ACT runs exp(g-1), and
                # the next unit's A-phase fills the previous unit's drain.
                # Norm: DVE pre-reduces E pairs -> quads so PE streams only
                # NLC/4 norm matmuls per unit, 2 chunks delayed so the DVE
                # adds never stall the PE queue.  PSUM accumulators (ot/nm,
                # single-buffered) are allocated lazily at first write so
                # the pool rotation lands after the previous unit's
                # evacuation instr is emitted.
                cur, ot_t, nm_t = {}, {}, {}
                etiles, ptiles, qtiles = {}, {}, {}
                for g in range(NG + 4 if do_any else 0):
                    if g < NG:
                        ug, cg = g // NLC, g % NLC
                        if cg == 0:
                            cur[ug] = loaded.pop(ug)
                        kt, vl, qt = cur[ug]
                        s = s_pool.tile([128, QSPAN], F32)
                        for half in range(2):
                            sl = bass.ts(half, 512)
                            nc.tensor.matmul(
                                s[:, sl], lhsT=kt[:, bass.ts(cg, 128)],
                                rhs=qt[:, sl], start=True, stop=True)
                        if do_exp:
                            e = e_pool.tile([128, QSPAN], BF16)
                            nc.scalar.activation(
                                e[:], s[:], mybir.ActivationFunctionType.Exp,
                                bias=bias_t[:, cg:cg + 1], scale=SCALE)
                            etiles[g] = e
                        else:
                            etiles[g] = dummy_e
                        if cg == 8 and ug + 1 < UNITS_PER_CORE:
                            load_unit(ug + 1)  # prefetch next unit's inputs
                    d = g - 1               # chunk whose B phase is due
                    if 0 <= d < NG and do_b:
                        ud, dl = d // NLC, d % NLC
                        if dl == 0:
                            ot_t[ud] = o_pool.tile([128, QSPAN], F32,
                                                   name=f"ot_u{ud}", tag="ot")
                        e = etiles[d]
                        for half in range(2):
                            sl = bass.ts(half, 512)
                            nc.tensor.matmul(
                                ot_t[ud][:, sl], lhsT=cur[ud][1][:, dl, :],
                                rhs=e[:, sl],
                                start=(dl == 0), stop=(dl == NLC - 1))
                        if do_exp and dl % 2 == 1:
                            pp = e_pool.tile([128, QSPAN], BF16,
                                             tag="pp", name=f"pp_{d}")
                            nc.vector.tensor_add(
                                pp[:], etiles.pop(d - 1)[:], etiles[d][:])
                            ptiles[d // 2] = pp
                        if do_exp and dl % 4 == 3:
                            qq = e_pool.tile([128, QSPAN], BF16,
                                             tag="qq", name=f"qq_{d}")
                            nc.vector.tensor_add(
                                qq[:], ptiles.pop(d // 2 - 1)[:],
                                ptiles.pop(d // 2)[:])
                            qtiles[d // 4] = qq
                        if not do_exp and dl % 4 == 3:
                            qtiles[d // 4] = dummy_e
                        if dl == NLC - 1:
                            etiles.pop(d)
                            ot = ot_t.pop(ud)
                            ot_sb = ob_pool.tile([128, QSPAN], F32,
                                                 name=f"otsb_u{ud}",
                                                 tag="otsb")
                            nc.vector.tensor_scalar_add(ot_sb[:], ot[:], 0.0)
                            nc.sync.dma_start(out=ot_ap[ud], in_=ot_sb[:])
                    n = g - 3               # norm quad due, 2 chunks delayed
                    if 0 <= n < NG and do_b:
                        un, nl = n // NLC, n % NLC
                        if nl % 4 == 3:
                            if nl == 3:
                                nm_t[un] = n_pool.tile([128, QSPAN], F32,
                                                       name=f"nm_u{un}",
                                                       tag="nm")
                            qq = qtiles.pop(n // 4)
                            for half in range(2):
                                sl = bass.ts(half, 512)
                                nc.tensor.matmul(
                                    nm_t[un][0:1, sl], lhsT=ones_t[:],
                                    rhs=qq[:, sl],
                                    start=(nl == 3), stop=(nl == NLC - 1))
                            if nl == NLC - 1:
                                nm = nm_t.pop(un)
                                nm_sb = ob_pool.tile([1, QSPAN], F32,
                                                     name=f"nmsb_u{un}",
                                                     tag="nmsb")
                                nc.vector.tensor_scalar_add(
                                    nm_sb[:], nm[0:1, :], 0.0)
                                nc.sync.dma_start(out=nm_ap[un], in_=nm_sb[:])

    nc.compile()
    return nc


def _get_program():
    global _CACHED
    if _CACHED is None:
        _CACHED = _build_program()
    return _CACHED


def _host_prep(q, k, v, frame_seqlen, current_block_start):
    fs = max(0, min(int(frame_seqlen), LK))
    bs = max(0, min(int(current_block_start), LK))
    logw = np.zeros(LK, np.float32)
    logw[fs:bs] = math.log(0.1)
    bias = np.ascontiguousarray(logw.reshape(NLC, 128).T)  # [128, NLC]

    q = np.asarray(q, dtype=np.float32)
    k = np.asarray(k, dtype=np.float32)
    v = np.asarray(v, dtype=np.float32)

    qT = np.ascontiguousarray(q[0].transpose(1, 2, 0)).astype(NP_BF16)  # [H,128,LQ]
    kT = np.ascontiguousarray(k[0].transpose(1, 2, 0)).astype(NP_BF16)  # [H,128,LK]
    vL = np.ascontiguousarray(v[0].transpose(1, 0, 2)).astype(NP_BF16)  # [H,LK,128]

    in_maps = []
    for i in range(N_CORES):
        units = [3 * i + uu for uu in range(UNITS_PER_CORE)]
        heads = [g // 2 for g in units]
        qhs = [g % 2 for g in units]
        in_maps.append({
            "qt": np.ascontiguousarray(
                np.stack([qT[h, :, qh * QSPAN:(qh + 1) * QSPAN]
                          for h, qh in zip(heads, qhs)])),
            "kt": np.ascontiguousarray(np.stack([kT[h] for h in heads])),
            "vl": np.ascontiguousarray(np.stack([vL[h] for h in heads])),
            "bias": bias,
        })
    return in_maps


def _assemble(results):
    out = np.empty((B, LQ, H, D), np.float32)
    for i in range(N_CORES):
        ot = results[i]["ot"]   # [3, 128, 1024] unnormalized O^T
        nm = results[i]["nm"][:, 0]   # [3, 1024]
        for uu in range(UNITS_PER_CORE):
            g = 3 * i + uu
            h, qh = g // 2, g % 2
            out[0, qh * QSPAN:(qh + 1) * QSPAN, h, :] = (
                ot[uu] / nm[uu][None, :]).T
    return out


def kernel(q, k, v, frame_seqlen, current_block_start):
    nc = _get_program()
    in_maps = _host_prep(q, k, v, frame_seqlen, current_block_start)
    res = run_bass_kernel_spmd(nc, in_maps, core_ids=list(range(N_CORES)))
    return _assemble(res.results)

